# revision 21
# baseline (speedup 1.0000x reference)
"""MixHop layer (gnn_message_passing) as a Trainium2 Bass/Tile SPMD kernel.

Math reformulation (A = sparse adjacency with edge weights, row=dst, col=src):
    x0 = x @ W0 + b0
    x1 = A @ (x @ W1 + b1) = (A @ x) @ W1 + d1 (x) b1      d1 = A @ 1
    x2 = A @ A @ (x @ W2 + b2) = (A @ A @ x) @ W2 + d2 (x) b2,  d2 = A @ d1
so only two sparse propagations of the raw features are needed:
    y1 = A @ x   (pass A),   y2 = A @ y1  (pass B, after all-gather of y1)

Sharding: destination rows are split across 8 cores (12544 rows each, padded
from 100000 to 100352). Edges are partitioned by destination shard and sorted
into 128-row destination blocks; block edge lists are split by source chunk
(dma_gather indices are int16, so sources are gathered from 4 chunks of
25088 rows) and padded to multiples of 128 ("edge tiles"). For each edge
tile the kernel:
  - gathers the 128 source rows (512B each) via dma_gather (one call per
    (stage, chunk), a stage being a group of consecutive dest blocks),
  - builds a [128 edges x 128 rows] selection matrix sel[e, r] =
    w[e] * (row_local[e] == r) with one DVE tensor_scalar op,
  - accumulates psum[rows, feat] += sel.T @ gathered on the tensor engine.
Per-(block, chunk) tile counts are padded to the max across cores so a
single SPMD program serves all 8 cores.
"""

import os
import sys

import numpy as np

for _p in ("/opt/trn_rl_repo",):
    if os.path.isdir(_p) and _p not in sys.path:
        sys.path.insert(0, _p)

import concourse.bacc as bacc
import concourse.bass as bass
import concourse.mybir as mybir
import concourse.tile as tile
from concourse.bass_utils import run_bass_kernel_spmd

F32 = mybir.dt.float32
I16 = mybir.dt.int16

N_CORES = 8
P = 128          # partitions / rows per block / edges per tile
NCHUNK = 4       # source chunks (int16 index reach)
STAGE_TILE_CAP = 80  # max edge tiles staged in SBUF at once


# ---------------------------------------------------------------------------
# host-side preprocessing
# ---------------------------------------------------------------------------

def _prep(x, row, col, edge_weight, n_cores=N_CORES):
    N, C = x.shape
    E = row.shape[0]
    S = -(-N // (n_cores * P)) * P          # shard rows per core
    NP = S * n_cores                        # padded node count
    B = S // P                              # dest blocks per core
    CH = NP // NCHUNK                       # source-chunk rows
    assert CH % P == 0 and CH <= 32768

    # weighted degrees (biases of hop1/hop2 after the reformulation)
    w64 = edge_weight.astype(np.float64)
    d1 = np.bincount(row, weights=w64, minlength=NP)[:NP]
    d2 = np.bincount(row, weights=w64 * d1[col], minlength=NP)[:NP]
    d1 = d1.astype(np.float32)
    d2 = d2.astype(np.float32)

    # sort edges by (destination block, source chunk)
    gblk = (row // P).astype(np.int64)
    chunk = (col // CH).astype(np.int64)
    order = np.lexsort((chunk, gblk))
    gblk_s = gblk[order]
    chunk_s = chunk[order]
    col_s = col[order].astype(np.int64)
    w_s = edge_weight[order].astype(np.float32)
    rloc_s = (row[order] % P).astype(np.float32)

    nblk = NP // P
    grp = gblk_s * NCHUNK + chunk_s                     # sorted group key
    cnt = np.bincount(grp, minlength=nblk * NCHUNK)     # edges per (gblk, c)
    tiles = -(-cnt // P)                                # ceil
    # uniform per-(local block, chunk) tile count across cores
    T_BC = tiles.reshape(n_cores, B, NCHUNK).max(axis=0).astype(np.int64)
    empty = T_BC.sum(axis=1) == 0
    T_BC[empty, 0] = 1                                  # >=1 tile per block
    T_B = T_BC.sum(axis=1)
    LT = int(T_B.sum())                                 # edge tiles/core/pass

    stages = _make_stages(T_B)

    # tile-order base of every (block, chunk): stage -> chunk -> block
    base_bc = np.zeros((B, NCHUNK), dtype=np.int64)
    pos0 = 0
    call_lens = []                                      # per stage: 4 lens
    for (b0, nb, _) in stages:
        lens = []
        for c in range(NCHUNK):
            n = 0
            for b in range(b0, b0 + nb):
                base_bc[b, c] = pos0
                pos0 += T_BC[b, c]
                n += T_BC[b, c]
            lens.append(int(n))
        call_lens.append(lens)
    assert pos0 == LT

    # final position of every edge
    grp_start = np.zeros(nblk * NCHUNK + 1, dtype=np.int64)
    np.cumsum(cnt, out=grp_start[1:])
    rank = np.arange(E, dtype=np.int64) - grp_start[grp]
    b_local = gblk_s % B
    e_core = gblk_s // B
    pos = P * base_bc[b_local, chunk_s] + rank

    L = P * LT
    eidx = np.zeros((n_cores, L), dtype=np.int16)       # pad -> chunk row 0
    erow = np.zeros((n_cores, L), dtype=np.float32)
    ew = np.zeros((n_cores, L), dtype=np.float32)       # pad -> weight 0
    eidx[e_core, pos] = (col_s - chunk_s * CH).astype(np.int16)
    erow[e_core, pos] = rloc_s
    ew[e_core, pos] = w_s

    # dma_gather index wrapping: idx i -> [i % 16, i // 16]; call bases are
    # multiples of 128 so a global wrap equals per-call wraps. Replicated to
    # all 8 Q7 core groups (16-partition stripes).
    eidx16 = np.ascontiguousarray(
        np.tile(eidx.reshape(n_cores, L // 16, 16).transpose(0, 2, 1),
                (1, 8, 1)))                             # [cores, 128, L//16]
    # edge j -> partition j%128, tile j//128
    erow = np.ascontiguousarray(erow.reshape(n_cores, LT, P).transpose(0, 2, 1))
    ew = np.ascontiguousarray(ew.reshape(n_cores, LT, P).transpose(0, 2, 1))

    x_pad = np.zeros((NP, C), dtype=np.float32)
    x_pad[:N] = x
    xT = np.ascontiguousarray(
        x_pad.reshape(n_cores, S, C).transpose(0, 2, 1))   # [cores, C, S]

    d1_sb = np.ascontiguousarray(
        d1.reshape(n_cores, B, P).transpose(0, 2, 1))      # [cores, 128, B]
    d2_sb = np.ascontiguousarray(
        d2.reshape(n_cores, B, P).transpose(0, 2, 1))

    return dict(N=N, C=C, S=S, NP=NP, B=B, CH=CH, T_BC=T_BC, T_B=T_B, LT=LT,
                stages=stages, call_lens=call_lens, base_bc=base_bc,
                eidx16=eidx16, erow=erow, ew=ew, x_pad=x_pad, xT=xT,
                d1=d1_sb, d2=d2_sb)


def _make_stages(T_B, cap=STAGE_TILE_CAP):
    """Group consecutive blocks into stages of <= cap tiles.
    Returns (first_block, n_blocks, stage_tile_offset) tuples."""
    stages = []
    b = 0
    off = 0
    B = len(T_B)
    while b < B:
        start = b
        soff = off
        tot = 0
        while b < B and (b == start or tot + T_B[b] <= cap):
            tot += int(T_B[b])
            off += int(T_B[b])
            b += 1
        stages.append((start, b - start, soff))
    return stages


# ---------------------------------------------------------------------------
# device program
# ---------------------------------------------------------------------------

def build_program(meta, n_cores=N_CORES, mode="full"):
    N, C, S, NP, B = meta["N"], meta["C"], meta["S"], meta["NP"], meta["B"]
    CH, T_BC, T_B, LT = meta["CH"], meta["T_BC"], meta["T_B"], meta["LT"]
    stages, call_lens, base_bc = (meta["stages"], meta["call_lens"],
                                  meta["base_bc"])
    ts_max = max(int(T_B[b0:b0 + nb].sum()) for b0, nb, _ in stages)

    nc = bacc.Bacc("TRN2", target_bir_lowering=False, debug=False,
                   num_devices=n_cores, num_swdge_queues=4)

    x_full = nc.dram_tensor("x_full", [NP, C], F32, kind="ExternalInput")
    xT_d = nc.dram_tensor("xT", [C, S], F32, kind="ExternalInput")
    wmat_d = nc.dram_tensor("wmat", [C, 3 * C], F32, kind="ExternalInput")
    consts_d = nc.dram_tensor("consts", [P, 5 * P + 2 * B], F32,
                              kind="ExternalInput")
    eidx_d = nc.dram_tensor("eidx", [P, (P * LT) // 16], I16,
                            kind="ExternalInput")
    erow_d = nc.dram_tensor("erow", [P, LT], F32, kind="ExternalInput")
    ew_d = nc.dram_tensor("ew", [P, LT], F32, kind="ExternalInput")
    out_d = nc.dram_tensor("out", [S, 3 * C], F32, kind="ExternalOutput")

    with tile.TileContext(nc) as tc:
        with (
            tc.tile_pool(name="dram", bufs=1, space="DRAM") as dram,
            tc.tile_pool(name="cpool", bufs=1) as cpool,
            tc.tile_pool(name="fpool", bufs=3) as fpool,
            tc.tile_pool(name="mpool", bufs=3) as mpool,
            tc.tile_pool(name="spool", bufs=8) as spool,
            tc.tile_pool(name="vpool", bufs=3) as vpool,
            tc.tile_pool(name="ypsum", bufs=3, space="PSUM") as ypsum,
            tc.tile_pool(name="tpsum", bufs=2, space="PSUM") as tpsum,
            tc.tile_pool(name="xpsum", bufs=3, space="PSUM") as xpsum,
        ):
            y1s = dram.tile([S, C], F32)                       # AG input
            y1f = dram.tile([NP, C], F32, addr_space="Shared")  # AG output

            # resident constants
            consts_t = cpool.tile([P, 5 * P + 2 * B], F32, tag="consts")
            nc.sync.dma_start(consts_t[:], consts_d[:])
            iota_t = consts_t[:, 0 * P:1 * P]
            eye_t = consts_t[:, 1 * P:2 * P]
            b0b_t = consts_t[:, 2 * P:3 * P]
            b1b_t = consts_t[:, 3 * P:4 * P]
            b2b_t = consts_t[:, 4 * P:5 * P]
            d1_t = consts_t[:, 5 * P:5 * P + B]
            d2_t = consts_t[:, 5 * P + B:5 * P + 2 * B]
            wmat_t = cpool.tile([C, 3 * C], F32, tag="wmat")
            nc.sync.dma_start(wmat_t[:], wmat_d[:])
            f_dummy = None
            if mode in ("compute_only", "scatter_only"):
                f_dummy = cpool.tile([P, ts_max, C], F32, tag="fdummy")
                nc.vector.memset(f_dummy[:], 0.0)
            w0_t = wmat_t[:, 0 * C:1 * C]
            w1_t = wmat_t[:, 1 * C:2 * C]
            w2_t = wmat_t[:, 2 * C:3 * C]

            NBMAX = max(nb for _, nb, _ in stages)

            def emit_pass(src_t, w_t, bias_bcast_t, d_col_t, out_col0,
                          writeback, x0_also, tscatter=False):
                for si, (b0, nb, soff) in enumerate(stages):
                    ts = int(T_B[b0:b0 + nb].sum())
                    # stage metadata loads
                    idx_t = mpool.tile([P, (P * ts_max) // 16], I16, tag="idx")
                    row_t = mpool.tile([P, ts_max], F32, tag="row")
                    w_e_t = mpool.tile([P, ts_max], F32, tag="we")
                    i16o = (P * soff) // 16
                    i16n = (P * ts) // 16
                    nc.sync.dma_start(idx_t[:, :i16n],
                                      eidx_d[:, i16o:i16o + i16n])
                    nc.sync.dma_start(row_t[:, :ts], erow_d[:, soff:soff + ts])
                    nc.sync.dma_start(w_e_t[:, :ts], ew_d[:, soff:soff + ts])
                    # gathers: one per source chunk
                    f_t = (f_dummy if mode in ("compute_only", "scatter_only")
                           else fpool.tile([P, ts_max, C], F32, tag="f"))
                    rel = 0
                    for c in range(0 if mode in ("compute_only", "scatter_only") else NCHUNK):
                        tsc = call_lens[si][c]
                        if tsc == 0:
                            continue
                        nidx = P * tsc
                        nc.gpsimd.dma_gather(
                            out_ap=f_t[:, rel:rel + tsc, :],
                            in_ap=x_full[c * CH:(c + 1) * CH, :]
                            if src_t is None else src_t[c * CH:(c + 1) * CH, :],
                            idxs_ap=idx_t[:, (P * rel) // 16:
                                          (P * rel) // 16 + nidx // 16],
                            num_idxs=nidx,
                            num_idxs_reg=nidx,
                            elem_size=C,
                            single_packet=False,
                            queue_num=c,
                        )
                        rel += tsc
                    if mode == "gather_only":
                        continue
                    # per-stage staging tiles (batched writes/loads)
                    nco = 2 * C if x0_also else C
                    x_st = vpool.tile([P, NBMAX, nco], F32, tag="xst")
                    y_st = (None if tscatter
                            else vpool.tile([P, NBMAX, C], F32, tag="yst"))
                    if x0_also:
                        xT_t = vpool.tile([C, NBMAX * P], F32, tag="xT")
                        nc.sync.dma_start(
                            xT_t[:, :nb * P],
                            xT_d[:, b0 * P:(b0 + nb) * P])
                    # per-block scatter matmuls + transforms
                    for b in range(b0, b0 + nb):
                        kk = b - b0
                        y_ps = ypsum.tile([P, C], F32, tag="ypsum")
                        tl = [(int(base_bc[b, c] - soff), int(T_BC[b, c]))
                              for c in range(NCHUNK) if T_BC[b, c] > 0]
                        ntile = sum(n for _, n in tl)
                        k = 0
                        for (g0, n) in tl:
                            for t in range(g0, g0 + n):
                                sel = spool.tile([P, P], F32, tag="sel")
                                nc.vector.tensor_scalar(
                                    out=sel[:],
                                    in0=iota_t,
                                    scalar1=row_t[:, t:t + 1],
                                    scalar2=w_e_t[:, t:t + 1],
                                    op0=mybir.AluOpType.is_equal,
                                    op1=mybir.AluOpType.mult,
                                )
                                if tscatter:
                                    # accumulate y^T directly: [c, rows]
                                    nc.tensor.matmul(
                                        out=y_ps[:],
                                        lhsT=f_t[:, t, :],
                                        rhs=sel[:],
                                        start=(k == 0),
                                        stop=(k == ntile - 1),
                                    )
                                else:
                                    nc.tensor.matmul(
                                        out=y_ps[:],
                                        lhsT=sel[:],
                                        rhs=f_t[:, t, :],
                                        start=(k == 0),
                                        stop=(k == ntile - 1),
                                    )
                                k += 1
                        if mode == "scatter_only":
                            continue

                        if tscatter:
                            yT_sb = vpool.tile([P, C], F32, tag="ytsb")
                            nc.vector.tensor_copy(yT_sb[:], y_ps[:])
                        else:
                            y_sb = y_st[:, kk, :]
                            nc.vector.tensor_copy(y_sb, y_ps[:])
                            # x_k = y @ W + d (x) b
                            yT_ps = tpsum.tile([P, C], F32, tag="tpsum")
                            nc.tensor.transpose(yT_ps[:], y_sb, eye_t)
                            yT_sb = vpool.tile([P, C], F32, tag="ytsb")
                            nc.vector.tensor_copy(yT_sb[:], yT_ps[:])
                        x_ps = xpsum.tile([P, C], F32, tag="xpsum")
                        nc.tensor.matmul(out=x_ps[:], lhsT=yT_sb[:], rhs=w_t,
                                         start=True, stop=True)
                        tmp = vpool.tile([P, C], F32, tag="tmp")
                        nc.vector.tensor_scalar(
                            out=tmp[:], in0=bias_bcast_t,
                            scalar1=d_col_t[:, b:b + 1], scalar2=None,
                            op0=mybir.AluOpType.mult,
                        )
                        xcol = C if x0_also else 0
                        nc.vector.tensor_tensor(
                            out=x_st[:, kk, xcol:xcol + C], in0=x_ps[:],
                            in1=tmp[:], op=mybir.AluOpType.add)

                        if x0_also:
                            x0_ps = xpsum.tile([P, C], F32, tag="xpsum")
                            nc.tensor.matmul(
                                out=x0_ps[:], lhsT=xT_t[:, kk * P:(kk + 1) * P],
                                rhs=w0_t, start=True, stop=True)
                            nc.vector.tensor_tensor(
                                out=x_st[:, kk, 0:C], in0=x0_ps[:], in1=b0b_t,
                                op=mybir.AluOpType.add)

                    if mode == "scatter_only":
                        continue
                    # batched per-stage stores
                    if writeback:
                        nc.sync.dma_start(
                            y1s[b0 * P:(b0 + nb) * P, :]
                            .rearrange("(g p) c -> p g c", p=P),
                            y_st[:, :nb, :])
                    nc.sync.dma_start(
                        out_d[b0 * P:(b0 + nb) * P, out_col0:out_col0 + nco]
                        .rearrange("(g p) c -> p g c", p=P),
                        x_st[:, :nb, :])

            # pass A: y1 = A @ x, x1 = y1 @ W1 + d1 (x) b1, plus x0 path
            # (writes out cols 0:256 = x0 | x1 in one batched store)
            emit_pass(None, w1_t, b1b_t, d1_t, 0, True, True)

            if mode not in ("scatter_only", "full_noag"):
              nc.gpsimd.collective_compute(
                "AllGather",
                mybir.AluOpType.bypass,
                replica_groups=[list(range(n_cores))],
                ins=[y1s[:].opt()],
                outs=[y1f[:].opt()],
            )

            # pass B: y2 = A @ y1, x2 = y2 @ W2 + d2 (x) b2
            # (transposed scatter: accumulates y2^T, skipping the PE
            #  transpose chain — y2 itself is never written back)
            emit_pass(y1f, w2_t, b2b_t, d2_t, 2 * C, False, False,
                      tscatter=True)

    nc.compile()
    return nc


# ---------------------------------------------------------------------------
# entry point
# ---------------------------------------------------------------------------

def make_in_maps(meta, W0, b0, W1, b1, W2, b2, n_cores=N_CORES):
    B = meta["B"]
    iota = np.tile(np.arange(P, dtype=np.float32), (P, 1))
    eye = np.eye(P, dtype=np.float32)
    b0b = np.tile(np.asarray(b0, np.float32), (P, 1))
    b1b = np.tile(np.asarray(b1, np.float32), (P, 1))
    b2b = np.tile(np.asarray(b2, np.float32), (P, 1))
    wmat = np.concatenate(
        [np.asarray(W0, np.float32), np.asarray(W1, np.float32),
         np.asarray(W2, np.float32)], axis=1)
    in_maps = []
    for c in range(n_cores):
        consts = np.concatenate(
            [iota, eye, b0b, b1b, b2b, meta["d1"][c], meta["d2"][c]], axis=1)
        in_maps.append({
            "x_full": meta["x_pad"],
            "xT": meta["xT"][c],
            "wmat": wmat,
            "consts": np.ascontiguousarray(consts),
            "eidx": meta["eidx16"][c],
            "erow": meta["erow"][c],
            "ew": meta["ew"][c],
        })
    return in_maps


def kernel(x, row, col, edge_weight, W0, b0, W1, b1, W2, b2):
    x = np.asarray(x, np.float32)
    row = np.asarray(row, np.int32)
    col = np.asarray(col, np.int32)
    edge_weight = np.asarray(edge_weight, np.float32)
    N = x.shape[0]

    meta = _prep(x, row, col, edge_weight)
    nc = build_program(meta)
    in_maps = make_in_maps(meta, W0, b0, W1, b1, W2, b2)
    res = run_bass_kernel_spmd(nc, in_maps, core_ids=list(range(N_CORES)))
    out = np.concatenate([r["out"] for r in res.results], axis=0)
    return np.ascontiguousarray(out[:N])


if __name__ == "__main__":
    rng = np.random.default_rng(0)
    N, C, E = 2048, 128, 8192
    x = rng.standard_normal((N, C), dtype=np.float32)
    row = rng.integers(0, N, E).astype(np.int32)
    col = rng.integers(0, N, E).astype(np.int32)
    w = rng.random(E, dtype=np.float32)
    meta = _prep(x, row, col, w)
    print("tiles/core/pass:", meta["LT"], "stages:", len(meta["stages"]))
